# revision 28
# baseline (speedup 1.0000x reference)
"""BackgroundNoiseLayer kernel for 8 trn2 NeuronCores.

Math: out[0, t, n] = sum_k W[n, k] * rest[t, k], where W [60000, 100] is
scatter-added from COO (v1 block rows 0..49999, lm block rows 50000..59999)
and the output feature axis is the concat of the two blocks.

Strategy (per sharding hint): densify the tiny sparse matrix host-side
(240k nnz -> dense W, ~0.002% of the matmul FLOPs), shard the post-synaptic
feature axis across the 8 cores (7500 features each), and run a dense
[1000,101] @ [101,7500] matmul per core. rest is tiny and replicated. Each
core writes its own contiguous output slice; concat on host is the no-op
gather.

Precision scheme (gate is rel_err < 2e-2): the device emits a per-feature
scaled int8 stream. Host folds 127/s_n into W (s_n = 5 sigma of feature n,
computed exactly from the actual rest moments) and appends a constant-1
column to rest carrying -127*mu_n/s_n, so PSUM holds the centered, scaled
value in [-127,127]. The copy out of PSUM casts f32->int8; host decodes
q*(s/127)+mu in f32. Measured rel err ~8.7e-3.

Pipeline (trace-tuned; baseline 56.5us local -> this version ~53.4-54.8):
- Three stations: PE (bf16 matmuls, 512-col chunks into [128,1024] f32 PSUM
  tiles x4 = all 8 banks), PSUM evacuation (DVE CAST ~1.22us + ACT
  activation-copy ~1.11us per 1024-tile, both 1x mode - the binding
  station, ~35.5us on ACT), out-DMA split across the sync HWDGE queue
  (stageA halves) and the gpsimd SWDGE queue (stageB halves).
- Hard constraints found by measurement (do not regress these):
  * DMA partition counts MUST be multiples of 16: [101,x] transfers crawl
    at ~20-40 GB/s vs ~190 for [112,x]. Hence KP=112 and ROWS=1008
    (7x128 + 112; rows 1000-1007 are junk the host drops).
  * The scalar HWDGE queue is unusable while ACT runs copies (transfers
    only progress in scalar-engine gaps; issues cost ~2.8us).
  * Block-0 w-chunk consumption (~0.54us per 229KB chunk) outruns one
    queue, so w chunks alternate sync/gpsimd in consumption order; DMA
    completion sems fire ~1.5-2us after last byte (write receipt).
  * End-of-kernel has a fixed ~9us epilogue (DMA receipt waits + a ~51
    per-sem reset cascade per engine, tensor slowest at ~127ns/reset).
    It is framework-emitted and does not scale with kernel structure;
    keep-warm dummy ops do NOT speed it up (tested).
- Copy split: DVE {0,2,4,6} on even blocks / {1,3,5} on odd (28 fulls);
  ACT gets the rest incl. all eight 332-tails (measured balanced, and
  tails-on-DVE variants measured worse).
- Last block ships chunk-aligned pieces ([0:2048] at j==1, [2048:4096] at
  j==3, [4096:6144] at j==5, rest at j==7) all on sync HWDGE, so only a
  1356-col piece is gated on the final copies and the receipt chain ends
  ~1us earlier.
- PE p-state: 5 warmup matmuls on a gpsimd-memset scratch bridge t0 to
  w0-arrival so the 1.2->2.4 GHz ramp (needs ~3us continuous busy) is not
  reset before the real stream starts.
"""

import os

import numpy as np

B, T = 1, 1000
NBKG = 100
NV1, NLM = 50000, 10000
NPOST = NV1 + NLM          # 60000
NCORES = 8
SHARD = NPOST // NCORES    # 7500 real features per core

KP = 112                   # contraction dim (100 real + 1 bias + pad to 7x16;
                           # partition counts that aren't multiples of 16 make
                           # DMA descriptor-gen fall off a cliff: [101,x] loads
                           # measured ~20 GB/s vs ~170 GB/s for [112,x])
ROWS = 1008                # time rows padded to 7x128 + 112 (multiple-of-16)
TBLK = 128                 # rows per full block
NT = 8                     # row blocks: 7 x 128 + 1 x 112
LAST_ROWS = ROWS - 7 * TBLK  # 112
MMN = 512                  # matmul free dim cap = one fp32 PSUM bank
DCW = 1024                 # chunk width (2 PSUM banks)
DCHUNKS = [(i * DCW, DCW) for i in range(7)] + [(7 * DCW, SHARD - 7 * DCW)]
HSPLIT = 4 * DCW           # 4096: stageA | stageB split
ALPHA = 5.0                # int8 scale: s_n = ALPHA * sigma_n

_compiled = None


def _build_module():
    import concourse.bacc as bacc
    import concourse.mybir as mybir
    import concourse.tile as tile

    f32 = mybir.dt.float32
    i8 = mybir.dt.int8
    bf16 = mybir.dt.bfloat16
    nc = bacc.Bacc("TRN2", target_bir_lowering=False, debug=False)
    restT = nc.dram_tensor("restT", [KP, ROWS], bf16, kind="ExternalInput")
    wT = nc.dram_tensor("wT", [KP, SHARD], bf16, kind="ExternalInput")
    out = nc.dram_tensor("out", [ROWS, SHARD], i8, kind="ExternalOutput")

    with tile.TileContext(nc) as tc:
        with (
            tc.tile_pool(name="inp", bufs=1) as inp,
            tc.tile_pool(name="stage", bufs=3) as stagep,
            tc.tile_pool(name="psum", bufs=4, space="PSUM") as psump,
        ):
            # Warmup scratch memset on DVE (its first CAST isn't until
            # ~t0+6.5, so a 0.5us memset at t0+1.5 is free; putting it on
            # gpsimd would delay the w1 SWDGE issue by ~0.7us, which gates
            # ACT's first copy). 5 dummy matmuls warm the PE HAM clock
            # gate while the first input DMAs are in flight.
            scratch = inp.tile([KP, 640], bf16, tag="warm")
            nc.vector.memset(scratch[:], 0.0)

            rest0 = inp.tile([KP, 2 * TBLK], bf16, tag="rest0")
            w_sb = []
            for j, (off, w) in enumerate(DCHUNKS):
                w_sb.append(inp.tile([KP, w], bf16, tag=f"w{j}", name=f"w{j}"))
            rest1 = inp.tile([KP, ROWS - 2 * TBLK], bf16, tag="rest1")

            # Input placement (trace-tuned): the scalar HWDGE queue is
            # unusable (transfers only progress in scalar-engine gaps; one
            # issue costs ~2.8us). The block-0 chunk consumption cadence
            # (~0.54us per 229KB w chunk) exceeds a single queue's input
            # rate, so w chunks alternate between the sync HWDGE and gpsimd
            # SWDGE queues in consumption order; each queue then only has
            # to deliver a chunk per ~1.1us.
            # rest0 (57KB) first, then w0 in two 512-col pieces: chunk 0's
            # first matmul becomes eligible after only rest0+w0a (~0.7us
            # earlier than waiting for the full 229KB w0).
            nc.sync.dma_start(rest0[:], restT[:, :2 * TBLK])
            nc.sync.dma_start(w_sb[0][:, :MMN], wT[:, 0:MMN])
            nc.sync.dma_start(w_sb[0][:, MMN:], wT[:, MMN:DCW])
            for j in (2, 4, 6):
                off, w = DCHUNKS[j]
                nc.sync.dma_start(w_sb[j][:], wT[:, off:off + w])
            for j in (1, 3, 5, 7):
                off, w = DCHUNKS[j]
                nc.gpsimd.dma_start(w_sb[j][:], wT[:, off:off + w])
            nc.gpsimd.dma_start(rest1[:], restT[:, 2 * TBLK:])

            for _ in range(5):
                psw = psump.tile([TBLK, DCW], f32, tag="ps")
                nc.tensor.matmul(psw[:, :MMN], scratch[:, :TBLK],
                                 scratch[:, TBLK:TBLK + MMN],
                                 start=True, stop=True)

            # Copy engine split (measured: DVE ~1.22us, ACT ~1.11us per
            # 1024-chunk; ACT gets all 332-tails + half the fulls).
            # Even blocks: DVE {0,2,4,6}; odd blocks: DVE {1,3,5}.
            for tb in range(NT):
                rows = TBLK if tb < 7 else LAST_ROWS
                r0 = tb * TBLK
                vector_chunks = {0, 2, 4, 6} if tb % 2 == 0 else {1, 3, 5}
                stageA = stagep.tile([TBLK, HSPLIT], i8, tag="stA",
                                     name=f"stA{tb}", bufs=3)
                stageB = stagep.tile([TBLK, SHARD - HSPLIT], i8, tag="stB",
                                     name=f"stB{tb}", bufs=3)
                if tb < 2:
                    lhsT = rest0[:, tb * TBLK:tb * TBLK + rows]
                else:
                    lhsT = rest1[:, (tb - 2) * TBLK:(tb - 2) * TBLK + rows]
                for j, (off, w) in enumerate(DCHUNKS):
                    ps = psump.tile([TBLK, DCW], f32, tag="ps")
                    for m in range((w + MMN - 1) // MMN):
                        n0 = m * MMN
                        n1 = min(w, n0 + MMN)
                        nc.tensor.matmul(
                            ps[:rows, n0:n1],
                            lhsT,
                            w_sb[j][:, n0:n1],
                            start=True,
                            stop=True,
                        )
                    copy = (nc.vector.tensor_copy if j in vector_chunks
                            else nc.scalar.copy)
                    if off < HSPLIT:
                        copy(stageA[:rows, off:off + w], ps[:rows, :w])
                    else:
                        copy(stageB[:rows, off - HSPLIT:off - HSPLIT + w],
                             ps[:rows, :w])
                    # stageA halves ride the sync HWDGE queue, stageB halves
                    # the gpsimd SWDGE queue: two independent DMA queues so
                    # the out stream drains at production rate. Last block
                    # goes out as quarters to shorten the final drain.
                    if j == 1 and tb == 7:
                        nc.sync.dma_start(out[r0:r0 + rows, :2 * DCW],
                                          stageA[:rows, :2 * DCW])
                    elif j == 3:
                        if tb < 7:
                            nc.sync.dma_start(out[r0:r0 + rows, :HSPLIT],
                                              stageA[:rows, :])
                        else:
                            nc.sync.dma_start(out[r0:r0 + rows, 2 * DCW:HSPLIT],
                                              stageA[:rows, 2 * DCW:])
                    elif j == 5 and tb == 7:
                        # last block: ship each piece as soon as it's
                        # copied so only the 332-col tail (37KB, ~0.2us)
                        # is gated on the final copy, shortening the
                        # end-of-kernel transfer+receipt chain. All on
                        # sync: HWDGE skips the ~3us SWDGE Q7 drain and
                        # has lower receipt latency.
                        nc.sync.dma_start(
                            out[r0:r0 + rows, HSPLIT:HSPLIT + 2 * DCW],
                            stageB[:rows, :2 * DCW])
                    elif j == 6 and tb == 7:
                        nc.sync.dma_start(
                            out[r0:r0 + rows, HSPLIT + 2 * DCW:HSPLIT + 3 * DCW],
                            stageB[:rows, 2 * DCW:3 * DCW])
                    elif j == 7:
                        if tb < 7:
                            nc.gpsimd.dma_start(out[r0:r0 + rows, HSPLIT:],
                                                stageB[:rows, :])
                        else:
                            nc.sync.dma_start(
                                out[r0:r0 + rows, HSPLIT + 3 * DCW:],
                                stageB[:rows, 3 * DCW:])

    nc.compile()
    return nc


def _densify(v1_weights, v1_rows, v1_cols, lm_weights, lm_rows, lm_cols):
    rows = np.concatenate([
        np.asarray(v1_rows).astype(np.int64),
        np.asarray(lm_rows).astype(np.int64) + NV1,
    ])
    cols = np.concatenate([
        np.asarray(v1_cols).astype(np.int64),
        np.asarray(lm_cols).astype(np.int64),
    ])
    w = np.concatenate([
        np.asarray(v1_weights, dtype=np.float32),
        np.asarray(lm_weights, dtype=np.float32),
    ])
    W = np.bincount(rows * NBKG + cols, weights=w, minlength=NPOST * NBKG)
    return W.astype(np.float32).reshape(NPOST, NBKG)


def kernel(rest, v1_weights, v1_rows, v1_cols, lm_weights, lm_rows, lm_cols):
    import ml_dtypes

    from concourse.bass_utils import run_bass_kernel_spmd

    bf16 = ml_dtypes.bfloat16

    global _compiled
    if _compiled is None:
        _compiled = _build_module()

    W = _densify(v1_weights, v1_rows, v1_cols, lm_weights, lm_rows, lm_cols)
    rest32 = np.asarray(rest, np.float32)

    # per-feature affine int8 code: psum = 127*(out - mu)/s, decoded
    # host-side as q*(s/127) + mu. mu and sigma are exact moments of the
    # actual rest sample, so s = ALPHA*sigma covers the deviations.
    lam = rest32.mean(0)                       # [NBKG]
    var = ((rest32 - lam) ** 2).mean(0)        # [NBKG]
    mu = W @ lam                               # [NPOST]
    sig = np.sqrt(np.maximum((W * W) @ var, 1e-12))
    s = ALPHA * sig
    Wq = W * (127.0 / s)[:, None]              # [NPOST, NBKG]
    muq = -127.0 * mu / s                      # [NPOST]

    restT = np.zeros((KP, ROWS), bf16)
    restT[:NBKG, :B * T] = rest32.astype(bf16).T
    restT[NBKG, :B * T] = bf16(1.0)            # bias column

    in_maps = []
    for c in range(NCORES):
        sl = slice(c * SHARD, (c + 1) * SHARD)
        wpad = np.zeros((KP, SHARD), bf16)
        wpad[:NBKG, :] = Wq[sl].T.astype(bf16)
        wpad[NBKG, :] = muq[sl].astype(bf16)
        in_maps.append({"restT": restT, "wT": wpad})

    trace = bool(int(os.environ.get("KERNEL_TRACE", "0")))
    if trace:
        _install_ntff_shim()
    res = run_bass_kernel_spmd(
        _compiled, in_maps, core_ids=list(range(NCORES)), trace=trace
    )
    kernel.last_results = res
    dec = [
        res.results[c]["out"][:B * T, :].astype(np.float32)
        * (s[c * SHARD:(c + 1) * SHARD] / 127.0)[None, :]
        + mu[c * SHARD:(c + 1) * SHARD][None, :]
        for c in range(NCORES)
    ]
    full = np.concatenate(dec, axis=1)
    return full.reshape(B, T, NPOST)


def _install_ntff_shim():
    """The agent image's antenv lacks axon_hooks; register the NTFF profile
    hook by dlopening libaxon_pjrt.so directly (same path trn_boot uses)."""
    import sys
    import types

    if "antenv.axon_hooks" in sys.modules:
        return
    try:
        from trn_agent_boot.trn_boot import _ntff_profile_via_ctypes

        hook = _ntff_profile_via_ctypes("/opt/axon/libaxon_pjrt.so")
    except Exception:
        hook = None
    mod = types.ModuleType("antenv.axon_hooks")
    mod.get_axon_ntff_profile_hook = lambda: hook
    mod.set_axon_ntff_profile_hook = lambda h: None
    sys.modules["antenv.axon_hooks"] = mod


# revision 29
# speedup vs baseline: 1.0088x; 1.0088x over previous
"""BackgroundNoiseLayer kernel for 8 trn2 NeuronCores.

Math: out[0, t, n] = sum_k W[n, k] * rest[t, k], where W [60000, 100] is
scatter-added from COO (v1 block rows 0..49999, lm block rows 50000..59999)
and the output feature axis is the concat of the two blocks.

Strategy (per sharding hint): densify the tiny sparse matrix host-side
(240k nnz -> dense W, ~0.002% of the matmul FLOPs), shard the post-synaptic
feature axis across the 8 cores (7500 features each), and run a dense
[1000,101] @ [101,7500] matmul per core. rest is tiny and replicated. Each
core writes its own contiguous output slice; concat on host is the no-op
gather.

Precision scheme (gate is rel_err < 2e-2): the device emits a per-feature
scaled int8 stream. Host folds 127/s_n into W (s_n = 5 sigma of feature n,
computed exactly from the actual rest moments) and appends a constant-1
column to rest carrying -127*mu_n/s_n, so PSUM holds the centered, scaled
value in [-127,127]. The copy out of PSUM casts f32->int8; host decodes
q*(s/127)+mu in f32. Measured rel err ~8.7e-3.

Pipeline (trace-tuned; baseline 56.5us local -> this version ~53.4-54.8):
- Three stations: PE (bf16 matmuls, 512-col chunks into [128,1024] f32 PSUM
  tiles x4 = all 8 banks), PSUM evacuation (DVE CAST ~1.22us + ACT
  activation-copy ~1.11us per 1024-tile, both 1x mode - the binding
  station, ~35.5us on ACT), out-DMA split across the sync HWDGE queue
  (stageA halves) and the gpsimd SWDGE queue (stageB halves).
- Hard constraints found by measurement (do not regress these):
  * DMA partition counts MUST be multiples of 16: [101,x] transfers crawl
    at ~20-40 GB/s vs ~190 for [112,x]. Hence KP=112 and ROWS=1008
    (7x128 + 112; rows 1000-1007 are junk the host drops).
  * The scalar HWDGE queue is unusable while ACT runs copies (transfers
    only progress in scalar-engine gaps; issues cost ~2.8us).
  * Block-0 w-chunk consumption (~0.54us per 229KB chunk) outruns one
    queue, so w chunks alternate sync/gpsimd in consumption order; DMA
    completion sems fire ~1.5-2us after last byte (write receipt).
  * End-of-kernel has a fixed ~9us epilogue (DMA receipt waits + a ~51
    per-sem reset cascade per engine, tensor slowest at ~127ns/reset).
    It is framework-emitted and does not scale with kernel structure;
    keep-warm dummy ops do NOT speed it up (tested).
- Copy split: DVE {0,2,4,6} on even blocks / {1,3,5} on odd (28 fulls);
  ACT gets the rest incl. all eight 332-tails (measured balanced, and
  tails-on-DVE variants measured worse).
- Last block ships chunk-aligned pieces ([0:2048] at j==1, [2048:4096] at
  j==3, [4096:6144] at j==5, rest at j==7) all on sync HWDGE, so only a
  1356-col piece is gated on the final copies and the receipt chain ends
  ~1us earlier.
- PE p-state: 5 warmup matmuls on a gpsimd-memset scratch bridge t0 to
  w0-arrival so the 1.2->2.4 GHz ramp (needs ~3us continuous busy) is not
  reset before the real stream starts.
"""

import os

import numpy as np

B, T = 1, 1000
NBKG = 100
NV1, NLM = 50000, 10000
NPOST = NV1 + NLM          # 60000
NCORES = 8
SHARD = NPOST // NCORES    # 7500 real features per core

KP = 112                   # contraction dim (100 real + 1 bias + pad to 7x16;
                           # partition counts that aren't multiples of 16 make
                           # DMA descriptor-gen fall off a cliff: [101,x] loads
                           # measured ~20 GB/s vs ~170 GB/s for [112,x])
ROWS = 1008                # time rows padded to 7x128 + 112 (multiple-of-16)
TBLK = 128                 # rows per full block
NT = 8                     # row blocks: 7 x 128 + 1 x 112
LAST_ROWS = ROWS - 7 * TBLK  # 112
MMN = 512                  # matmul free dim cap = one fp32 PSUM bank
DCW = 1024                 # chunk width (2 PSUM banks)
DCHUNKS = [(i * DCW, DCW) for i in range(7)] + [(7 * DCW, SHARD - 7 * DCW)]
HSPLIT = 4 * DCW           # 4096: stageA | stageB split
ALPHA = 5.0                # int8 scale: s_n = ALPHA * sigma_n

_compiled = None


def _build_module():
    import concourse.bacc as bacc
    import concourse.mybir as mybir
    import concourse.tile as tile

    f32 = mybir.dt.float32
    i8 = mybir.dt.int8
    bf16 = mybir.dt.bfloat16
    nc = bacc.Bacc("TRN2", target_bir_lowering=False, debug=False)
    restT = nc.dram_tensor("restT", [KP, ROWS], bf16, kind="ExternalInput")
    wT = nc.dram_tensor("wT", [KP, SHARD], bf16, kind="ExternalInput")
    out = nc.dram_tensor("out", [ROWS, SHARD], i8, kind="ExternalOutput")

    with tile.TileContext(nc) as tc:
        with (
            tc.tile_pool(name="inp", bufs=1) as inp,
            tc.tile_pool(name="stage", bufs=3) as stagep,
            tc.tile_pool(name="psum", bufs=4, space="PSUM") as psump,
        ):
            # Warmup scratch memset on DVE (its first CAST isn't until
            # ~t0+6.5, so a 0.5us memset at t0+1.5 is free; putting it on
            # gpsimd would delay the w1 SWDGE issue by ~0.7us, which gates
            # ACT's first copy). 5 dummy matmuls warm the PE HAM clock
            # gate while the first input DMAs are in flight.
            scratch = inp.tile([KP, 640], bf16, tag="warm")
            nc.vector.memset(scratch[:], 0.0)

            rest0 = inp.tile([KP, 2 * TBLK], bf16, tag="rest0")
            w_sb = []
            for j, (off, w) in enumerate(DCHUNKS):
                w_sb.append(inp.tile([KP, w], bf16, tag=f"w{j}", name=f"w{j}"))
            rest1 = inp.tile([KP, ROWS - 2 * TBLK], bf16, tag="rest1")

            # Input placement (trace-tuned): the scalar HWDGE queue is
            # unusable (transfers only progress in scalar-engine gaps; one
            # issue costs ~2.8us). The block-0 chunk consumption cadence
            # (~0.54us per 229KB w chunk) exceeds a single queue's input
            # rate, so w chunks alternate between the sync HWDGE and gpsimd
            # SWDGE queues in consumption order; each queue then only has
            # to deliver a chunk per ~1.1us.
            # (Tested: splitting w0 into 512-col pieces with rest0 first
            # starts chunk 0 earlier but delays w2/w4 by the extra issue,
            # moving the stall rather than removing it.)
            nc.sync.dma_start(w_sb[0][:], wT[:, 0:DCW])
            nc.sync.dma_start(rest0[:], restT[:, :2 * TBLK])
            for j in (2, 4, 6):
                off, w = DCHUNKS[j]
                nc.sync.dma_start(w_sb[j][:], wT[:, off:off + w])
            for j in (1, 3, 5, 7):
                off, w = DCHUNKS[j]
                nc.gpsimd.dma_start(w_sb[j][:], wT[:, off:off + w])
            nc.gpsimd.dma_start(rest1[:], restT[:, 2 * TBLK:])

            for _ in range(5):
                psw = psump.tile([TBLK, DCW], f32, tag="ps")
                nc.tensor.matmul(psw[:, :MMN], scratch[:, :TBLK],
                                 scratch[:, TBLK:TBLK + MMN],
                                 start=True, stop=True)

            # Copy engine split (measured: DVE ~1.22us, ACT ~1.11us per
            # 1024-chunk; ACT gets all 332-tails + half the fulls).
            # Even blocks: DVE {0,2,4,6}; odd blocks: DVE {1,3,5}.
            for tb in range(NT):
                rows = TBLK if tb < 7 else LAST_ROWS
                r0 = tb * TBLK
                vector_chunks = {0, 2, 4, 6} if tb % 2 == 0 else {1, 3, 5}
                stageA = stagep.tile([TBLK, HSPLIT], i8, tag="stA",
                                     name=f"stA{tb}", bufs=3)
                stageB = stagep.tile([TBLK, SHARD - HSPLIT], i8, tag="stB",
                                     name=f"stB{tb}", bufs=3)
                if tb < 2:
                    lhsT = rest0[:, tb * TBLK:tb * TBLK + rows]
                else:
                    lhsT = rest1[:, (tb - 2) * TBLK:(tb - 2) * TBLK + rows]
                for j, (off, w) in enumerate(DCHUNKS):
                    ps = psump.tile([TBLK, DCW], f32, tag="ps")
                    for m in range((w + MMN - 1) // MMN):
                        n0 = m * MMN
                        n1 = min(w, n0 + MMN)
                        nc.tensor.matmul(
                            ps[:rows, n0:n1],
                            lhsT,
                            w_sb[j][:, n0:n1],
                            start=True,
                            stop=True,
                        )
                    copy = (nc.vector.tensor_copy if j in vector_chunks
                            else nc.scalar.copy)
                    if off < HSPLIT:
                        copy(stageA[:rows, off:off + w], ps[:rows, :w])
                    else:
                        copy(stageB[:rows, off - HSPLIT:off - HSPLIT + w],
                             ps[:rows, :w])
                    # stageA halves ride the sync HWDGE queue, stageB halves
                    # the gpsimd SWDGE queue: two independent DMA queues so
                    # the out stream drains at production rate. Last block
                    # goes out as quarters to shorten the final drain.
                    if j == 1 and tb == 7:
                        nc.sync.dma_start(out[r0:r0 + rows, :2 * DCW],
                                          stageA[:rows, :2 * DCW])
                    elif j == 3:
                        if tb < 7:
                            nc.sync.dma_start(out[r0:r0 + rows, :HSPLIT],
                                              stageA[:rows, :])
                        else:
                            nc.sync.dma_start(out[r0:r0 + rows, 2 * DCW:HSPLIT],
                                              stageA[:rows, 2 * DCW:])
                    elif j == 5 and tb == 7:
                        # last block: ship each piece as soon as it's
                        # copied so only the 332-col tail (37KB, ~0.2us)
                        # is gated on the final copy, shortening the
                        # end-of-kernel transfer+receipt chain. All on
                        # sync: HWDGE skips the ~3us SWDGE Q7 drain and
                        # has lower receipt latency.
                        nc.sync.dma_start(
                            out[r0:r0 + rows, HSPLIT:HSPLIT + 2 * DCW],
                            stageB[:rows, :2 * DCW])
                    elif j == 6 and tb == 7:
                        nc.sync.dma_start(
                            out[r0:r0 + rows, HSPLIT + 2 * DCW:HSPLIT + 3 * DCW],
                            stageB[:rows, 2 * DCW:3 * DCW])
                    elif j == 7:
                        if tb < 7:
                            nc.gpsimd.dma_start(out[r0:r0 + rows, HSPLIT:],
                                                stageB[:rows, :])
                        else:
                            nc.sync.dma_start(
                                out[r0:r0 + rows, HSPLIT + 3 * DCW:],
                                stageB[:rows, 3 * DCW:])

    nc.compile()
    return nc


def _densify(v1_weights, v1_rows, v1_cols, lm_weights, lm_rows, lm_cols):
    rows = np.concatenate([
        np.asarray(v1_rows).astype(np.int64),
        np.asarray(lm_rows).astype(np.int64) + NV1,
    ])
    cols = np.concatenate([
        np.asarray(v1_cols).astype(np.int64),
        np.asarray(lm_cols).astype(np.int64),
    ])
    w = np.concatenate([
        np.asarray(v1_weights, dtype=np.float32),
        np.asarray(lm_weights, dtype=np.float32),
    ])
    W = np.bincount(rows * NBKG + cols, weights=w, minlength=NPOST * NBKG)
    return W.astype(np.float32).reshape(NPOST, NBKG)


def kernel(rest, v1_weights, v1_rows, v1_cols, lm_weights, lm_rows, lm_cols):
    import ml_dtypes

    from concourse.bass_utils import run_bass_kernel_spmd

    bf16 = ml_dtypes.bfloat16

    global _compiled
    if _compiled is None:
        _compiled = _build_module()

    W = _densify(v1_weights, v1_rows, v1_cols, lm_weights, lm_rows, lm_cols)
    rest32 = np.asarray(rest, np.float32)

    # per-feature affine int8 code: psum = 127*(out - mu)/s, decoded
    # host-side as q*(s/127) + mu. mu and sigma are exact moments of the
    # actual rest sample, so s = ALPHA*sigma covers the deviations.
    lam = rest32.mean(0)                       # [NBKG]
    var = ((rest32 - lam) ** 2).mean(0)        # [NBKG]
    mu = W @ lam                               # [NPOST]
    sig = np.sqrt(np.maximum((W * W) @ var, 1e-12))
    s = ALPHA * sig
    Wq = W * (127.0 / s)[:, None]              # [NPOST, NBKG]
    muq = -127.0 * mu / s                      # [NPOST]

    restT = np.zeros((KP, ROWS), bf16)
    restT[:NBKG, :B * T] = rest32.astype(bf16).T
    restT[NBKG, :B * T] = bf16(1.0)            # bias column

    in_maps = []
    for c in range(NCORES):
        sl = slice(c * SHARD, (c + 1) * SHARD)
        wpad = np.zeros((KP, SHARD), bf16)
        wpad[:NBKG, :] = Wq[sl].T.astype(bf16)
        wpad[NBKG, :] = muq[sl].astype(bf16)
        in_maps.append({"restT": restT, "wT": wpad})

    trace = bool(int(os.environ.get("KERNEL_TRACE", "0")))
    if trace:
        _install_ntff_shim()
    res = run_bass_kernel_spmd(
        _compiled, in_maps, core_ids=list(range(NCORES)), trace=trace
    )
    kernel.last_results = res
    dec = [
        res.results[c]["out"][:B * T, :].astype(np.float32)
        * (s[c * SHARD:(c + 1) * SHARD] / 127.0)[None, :]
        + mu[c * SHARD:(c + 1) * SHARD][None, :]
        for c in range(NCORES)
    ]
    full = np.concatenate(dec, axis=1)
    return full.reshape(B, T, NPOST)


def _install_ntff_shim():
    """The agent image's antenv lacks axon_hooks; register the NTFF profile
    hook by dlopening libaxon_pjrt.so directly (same path trn_boot uses)."""
    import sys
    import types

    if "antenv.axon_hooks" in sys.modules:
        return
    try:
        from trn_agent_boot.trn_boot import _ntff_profile_via_ctypes

        hook = _ntff_profile_via_ctypes("/opt/axon/libaxon_pjrt.so")
    except Exception:
        hook = None
    mod = types.ModuleType("antenv.axon_hooks")
    mod.get_axon_ntff_profile_hook = lambda: hook
    mod.set_axon_ntff_profile_hook = lambda h: None
    sys.modules["antenv.axon_hooks"] = mod
